# revision 5
# baseline (speedup 1.0000x reference)
"""Trainium2 Bass kernel for the BalancedSpikingNetwork problem.

Strategy: model-parallel over neurons across 8 NeuronCores.
  - Each core owns 256 E-neurons + 64 I-neurons (padded to 384 = 3x128 rows).
  - Per step: 24 gathered spike chunks + 1 local input chunk accumulate into a
    PSUM tile [64, 320] = input currents for this core's neurons (batch-major).
  - LIF update with fused scalar_tensor_tensor ops; spikes are produced in
    [neuron, batch] layout via PE transposes and exchanged with an AllGather.
  - Spike rate sums accumulate on-chip; final readout matmul happens on host.

The spike at step t depends only on state through t-1 (v_dec = v + dt*(i - v)
is computed before the step-t input current lands), so the AllGather of z(t)
overlaps with the step-t matmuls.
"""

import sys

for _p in ("/opt/trn_rl_repo", "/root/.axon_site/_ro/trn_rl_repo"):
    if _p not in sys.path:
        sys.path.append(_p)

import numpy as np

import concourse.bass as bass
import concourse.mybir as mybir
import concourse.tile as tile
from concourse import bacc
from concourse.bass_utils import run_bass_kernel_spmd
from concourse.masks import make_identity

F32 = mybir.dt.float32
OP = mybir.AluOpType

B, T_FULL, IN = 64, 512, 128
N_E, N_I = 2048, 512
NCORES = 8
E_LOC = N_E // NCORES          # 256
I_LOC = N_I // NCORES          # 64
NLOC = E_LOC + I_LOC           # 320 real outputs per core
PADLOC = 384                   # padded to 3 chunks of 128
NCHUNK = PADLOC // 128         # 3 chunks per source core
GCHUNK = NCORES * NCHUNK       # 24 gathered spike chunks
KSRC = GCHUNK * 128 + IN       # 3200 combined contraction rows (incl input)

TAU_E = 1.0 / 20.0
TAU_I = 1.0 / 10.0
SYN_DEC = 1.0 - 1.0 / 5.0      # 0.8


def build_kernel(T: int, trace_scopes: bool = False):
    nc = bacc.Bacc(
        "TRN2", target_bir_lowering=False, debug=False, num_devices=NCORES
    )

    W_in = nc.dram_tensor("W", [KSRC, NLOC], F32, kind="ExternalInput")
    XT_in = nc.dram_tensor("XT", [T, IN, B], F32, kind="ExternalInput")
    RATES_out = nc.dram_tensor("RATES", [128, 2 * B], F32, kind="ExternalOutput")

    rg = [list(range(NCORES))]

    with tile.TileContext(nc) as tc:
        with (
            tc.tile_pool(name="persist", bufs=1) as pp,
            tc.tile_pool(name="step", bufs=2) as sp,
            tc.tile_pool(name="psum", bufs=2, space="PSUM") as psp,
            tc.tile_pool(name="tpsum", bufs=1, space="PSUM") as tpp,
            tc.tile_pool(name="dram", bufs=2, space="DRAM") as dp,
        ):
            # --- persistent tiles ---
            w_sb = pp.tile([128, (GCHUNK + 1) * NLOC], F32)       # weights
            v_sb = pp.tile([B, NLOC], F32)                        # membrane
            u_sb = pp.tile([B, NLOC], F32)                        # syn current
            zt_sb = pp.tile([128, NCHUNK * B], F32)               # spike staging [n, b]
            rates_sb = pp.tile([128, 2 * B], F32)                 # E spike-count [n, b]
            ident = pp.tile([B, B], F32)
            bvec = pp.tile([B, NLOC], F32)                        # per-col tau scale

            # weights: 25 chunk loads [128, 320] each
            for k in range(GCHUNK + 1):
                nc.sync.dma_start(
                    out=w_sb[:, k * NLOC : (k + 1) * NLOC],
                    in_=W_in[k * 128 : (k + 1) * 128, :],
                )
            make_identity(nc, ident)
            nc.vector.memset(v_sb, 0.0)
            nc.vector.memset(u_sb, 0.0)
            nc.gpsimd.memset(zt_sb, 0.0)
            nc.gpsimd.memset(rates_sb, 0.0)
            nc.gpsimd.memset(bvec[:, :E_LOC], TAU_E)
            nc.gpsimd.memset(bvec[:, E_LOC:], TAU_I)

            ag_prev = None  # DRAM tile holding gathered spikes of step t-1

            for t in range(T):
                # ---- input currents: 25 (or 1) accumulating matmuls ----
                s_t = sp.tile([128, (GCHUNK + 1) * B], F32, tag="S")
                nc.sync.dma_start(
                    out=s_t[:, GCHUNK * B : (GCHUNK + 1) * B], in_=XT_in[t]
                )
                psum = psp.tile([B, NLOC], F32, tag="PS")
                nc.tensor.matmul(
                    psum,
                    s_t[:, GCHUNK * B : (GCHUNK + 1) * B],
                    w_sb[:, GCHUNK * NLOC : (GCHUNK + 1) * NLOC],
                    start=True,
                    stop=(ag_prev is None),
                )
                if ag_prev is not None:
                    # load gathered spikes: 8 tiles [128, 192] from DRAM
                    for d in range(NCORES):
                        nc.sync.dma_start(
                            out=s_t[:, d * NCHUNK * B : (d + 1) * NCHUNK * B],
                            in_=ag_prev[d * 128 : (d + 1) * 128, :],
                        )
                    for k in range(GCHUNK):
                        nc.tensor.matmul(
                            psum,
                            s_t[:, k * B : (k + 1) * B],
                            w_sb[:, k * NLOC : (k + 1) * NLOC],
                            start=False,
                            stop=(k == GCHUNK - 1),
                        )

                # ---- v_dec from prior state (no dependence on psum) ----
                v_dec = sp.tile([B, NLOC], F32, tag="VD")
                nc.vector.scalar_tensor_tensor(
                    out=v_dec[:, :E_LOC], in0=v_sb[:, :E_LOC], scalar=1.0 - TAU_E,
                    in1=u_sb[:, :E_LOC], op0=OP.mult, op1=OP.add,
                )
                nc.vector.scalar_tensor_tensor(
                    out=v_dec[:, E_LOC:], in0=v_sb[:, E_LOC:], scalar=1.0 - TAU_I,
                    in1=u_sb[:, E_LOC:], op0=OP.mult, op1=OP.add,
                )

                # ---- spikes in [n, b] layout: transpose + threshold ----
                for j in range(NCHUNK):
                    w = 128 if j < 2 else I_LOC
                    tp = tpp.tile([128, B], F32, tag=f"TP{j}")
                    nc.tensor.transpose(
                        tp[:w, :], v_dec[:, j * 128 : j * 128 + w], ident
                    )
                    nc.vector.tensor_scalar(
                        out=zt_sb[:w, j * B : (j + 1) * B], in0=tp[:w, :],
                        scalar1=1.0, scalar2=None, op0=OP.is_gt,
                    )

                # ---- exchange spikes (overlaps with everything below) ----
                if 1 <= t <= T - 2:
                    ag_in = dp.tile([128, NCHUNK * B], F32, tag="AGI")
                    ag_out = dp.tile([NCORES * 128, NCHUNK * B], F32, tag="AGO")
                    nc.sync.dma_start(out=ag_in[:, : 2 * B], in_=zt_sb[:, : 2 * B])
                    nc.sync.dma_start(out=ag_in[:, 2 * B :], in_=zt_sb[:, 2 * B :])
                    nc.gpsimd.collective_compute(
                        "AllGather",
                        OP.bypass,
                        replica_groups=rg,
                        ins=[ag_in[:]],
                        outs=[ag_out[:]],
                    )
                    ag_prev = ag_out
                else:
                    ag_prev = None if t == 0 else ag_prev

                # ---- rates accumulation (E rows only) ----
                nc.gpsimd.tensor_tensor(
                    out=rates_sb, in0=rates_sb, in1=zt_sb[:, : 2 * B], op=OP.add
                )

                # ---- state updates ----
                # v = (v_dec <= 1) * v_dec   (Pool engine, off the z critical path)
                nz = sp.tile([B, NLOC], F32, tag="NZ")
                nc.gpsimd.tensor_scalar(
                    out=nz, in0=v_dec, scalar1=1.0, scalar2=None, op0=OP.is_le
                )
                nc.gpsimd.tensor_tensor(out=v_sb, in0=v_dec, in1=nz, op=OP.mult)
                # u = 0.8*u + bvec * psum
                u08 = sp.tile([B, NLOC], F32, tag="U8")
                nc.scalar.activation(
                    out=u08, in_=u_sb, func=mybir.ActivationFunctionType.Copy,
                    scale=SYN_DEC,
                )
                ub = sp.tile([B, NLOC], F32, tag="UB")
                nc.vector.tensor_tensor(out=ub, in0=psum, in1=bvec, op=OP.mult)
                nc.gpsimd.tensor_tensor(out=u_sb, in0=ub, in1=u08, op=OP.add)

            nc.sync.dma_start(out=RATES_out[:], in_=rates_sb[:])

    nc.compile()
    return nc


def _prep_inputs(x, W_ee, W_ie, W_ei, W_ii, W_e_in, W_i_in):
    """Host-side: combined per-core weight matrices + transposed input."""
    Wee = np.maximum(W_ee, 0).astype(np.float32)
    Wie = np.maximum(W_ie, 0).astype(np.float32)
    Wei = np.maximum(W_ei, 0).astype(np.float32)
    Wii = np.maximum(W_ii, 0).astype(np.float32)

    Ws = []
    for c in range(NCORES):
        Ec = slice(c * E_LOC, (c + 1) * E_LOC)
        Ic = slice(c * I_LOC, (c + 1) * I_LOC)
        Wc = np.zeros((KSRC, NLOC), np.float32)
        for d in range(NCORES):
            base = d * PADLOC
            Epre = slice(d * E_LOC, (d + 1) * E_LOC)
            Ipre = slice(d * I_LOC, (d + 1) * I_LOC)
            Wc[base : base + E_LOC, :E_LOC] = Wee[Ec, Epre].T
            Wc[base : base + E_LOC, E_LOC:] = Wie[Ic, Epre].T
            Wc[base + E_LOC : base + NLOC, :E_LOC] = -Wei[Ec, Ipre].T
            Wc[base + E_LOC : base + NLOC, E_LOC:] = -Wii[Ic, Ipre].T
        Wc[GCHUNK * 128 :, :E_LOC] = W_e_in[Ec].T
        Wc[GCHUNK * 128 :, E_LOC:] = W_i_in[Ic].T
        Ws.append(Wc)

    xT = np.ascontiguousarray(
        np.asarray(x, np.float32).transpose(1, 2, 0)
    )  # [T, IN, B]
    return Ws, xT


_CACHE = {}


def _get_kernel(T):
    if T not in _CACHE:
        _CACHE[T] = build_kernel(T)
    return _CACHE[T]


def run_spikes(x, W_ee, W_ie, W_ei, W_ii, W_e_in, W_i_in, T=None, trace=False):
    """Run the device portion; returns spike-rate sums [B, N_E] (not yet /T)
    and the BassKernelResults."""
    T = x.shape[1] if T is None else T
    Ws, xT = _prep_inputs(x, W_ee, W_ie, W_ei, W_ii, W_e_in, W_i_in)
    xT = xT[:T]
    nc = _get_kernel(T)
    in_maps = [{"W": Ws[c], "XT": xT} for c in range(NCORES)]
    res = run_bass_kernel_spmd(
        nc, in_maps, core_ids=list(range(NCORES)), trace=trace
    )
    R = np.stack([res.results[c]["RATES"] for c in range(NCORES)])  # [c,p,2B]
    R = R.reshape(NCORES, 128, 2, B)                   # [c, p, j, b]
    counts = R.transpose(3, 0, 2, 1).reshape(B, N_E)   # [b, c*256+j*128+p]
    return counts, res


def kernel(x, W_ee, W_ie, W_ei, W_ii, W_e_in, W_i_in, readout_w, readout_b):
    counts, _ = run_spikes(x, W_ee, W_ie, W_ei, W_ii, W_e_in, W_i_in)
    rates = counts / np.float32(x.shape[1])
    y = rates.astype(np.float32) @ np.asarray(readout_w, np.float32).T
    return (y + np.asarray(readout_b, np.float32)).astype(np.float32)


# revision 9
# speedup vs baseline: 1.7128x; 1.7128x over previous
"""Trainium2 Bass kernel for the BalancedSpikingNetwork problem.

Strategy: model-parallel over neurons across 8 NeuronCores.
  - Each core owns 256 E-neurons + 64 I-neurons (padded to 384 = 3x128 rows).
  - Per step: 24 gathered spike chunks (bf16) + 1 local input chunk (fp32)
    accumulate into a PSUM tile [64, 320] = tau-scaled input currents for this
    core's neurons (batch-major). Weights are pre-scaled by tau on the host so
    the LIF update needs no per-region scaling of the current.
  - LIF update with fused scalar_tensor_tensor ops; spikes are produced in
    [neuron, batch] layout via PE transposes and exchanged with an AllGather.
  - Spike rate sums accumulate on-chip; final readout matmul happens on host.

The spike at step t depends only on state through t-1 (v_dec = v + dt*(i - v)
is computed before the step-t input current lands), so the AllGather of z(t)
overlaps with the step-t matmuls.
"""

import sys

for _p in ("/opt/trn_rl_repo", "/root/.axon_site/_ro/trn_rl_repo"):
    if _p not in sys.path:
        sys.path.append(_p)

import numpy as np
import ml_dtypes

import concourse.bass as bass
import concourse.mybir as mybir
import concourse.tile as tile
from concourse import bacc
from concourse.bass_utils import run_bass_kernel_spmd
from concourse.masks import make_identity

F32 = mybir.dt.float32
BF16 = mybir.dt.bfloat16
F32R = mybir.dt.float32r
import os
MM_MODE = os.environ.get("MM_DT", "f32r")
MM_DT = {"bf16": BF16, "f32r": F32R, "f32": F32}[MM_MODE]
MM_NP = {"bf16": ml_dtypes.bfloat16, "f32r": np.float32, "f32": np.float32}[MM_MODE]


def _f32(ap):
    """View a (possibly f32r) AP as plain f32 for non-matmul engines."""
    return ap.bitcast(F32) if MM_MODE == "f32r" else ap
OP = mybir.AluOpType

B, T_FULL, IN = 64, 512, 128
N_E, N_I = 2048, 512
NCORES = 8
E_LOC = N_E // NCORES          # 256
I_LOC = N_I // NCORES          # 64
NLOC = E_LOC + I_LOC           # 320 real outputs per core
PADLOC = 384                   # padded to 3 chunks of 128
NCHUNK = PADLOC // 128         # 3 chunks per source core
GCHUNK = NCORES * NCHUNK       # 24 gathered spike chunks
KSRC = GCHUNK * 128            # 3072 gathered contraction rows

TAU_E = 1.0 / 20.0
TAU_I = 1.0 / 10.0
SYN_DEC = 1.0 - 1.0 / 5.0      # 0.8


def build_kernel(T: int):
    nc = bacc.Bacc(
        "TRN2", target_bir_lowering=False, debug=False, num_devices=NCORES
    )

    W_in = nc.dram_tensor("W", [KSRC, NLOC], MM_DT, kind="ExternalInput")
    WIN_in = nc.dram_tensor("WIN", [IN, NLOC], F32, kind="ExternalInput")
    XT_in = nc.dram_tensor("XT", [T, IN, B], F32, kind="ExternalInput")
    RATES_out = nc.dram_tensor("RATES", [128, 2 * B], F32, kind="ExternalOutput")

    rg = [list(range(NCORES))]

    with tile.TileContext(nc) as tc:
        with (
            tc.tile_pool(name="persist", bufs=1) as pp,
            tc.tile_pool(name="step", bufs=2) as sp,
            tc.tile_pool(name="psum", bufs=2, space="PSUM") as psp,
            tc.tile_pool(name="tpsum", bufs=1, space="PSUM") as tpp,
            tc.tile_pool(name="dram", bufs=2, space="DRAM") as dp,
        ):
            # --- persistent tiles ---
            w_sb = pp.tile([128, GCHUNK * NLOC], MM_DT)            # recurrent wts
            win_sb = pp.tile([128, NLOC], F32)                    # input weights
            v_sb = pp.tile([B, NLOC], F32)                        # membrane
            u_sb = pp.tile([B, NLOC], F32)                        # tau*syn current
            zt_sb = pp.tile([128, NCHUNK * B], MM_DT)              # spikes [n, b]
            rates_sb = pp.tile([128, 2 * B], F32)                 # E counts [n, b]
            ident = pp.tile([B, B], F32)

            for k in range(GCHUNK):
                nc.sync.dma_start(
                    out=w_sb[:, k * NLOC : (k + 1) * NLOC],
                    in_=W_in[k * 128 : (k + 1) * 128, :],
                )
            nc.sync.dma_start(out=win_sb, in_=WIN_in[:])
            make_identity(nc, ident)
            nc.vector.memset(v_sb, 0.0)
            nc.vector.memset(u_sb, 0.0)
            nc.gpsimd.memset(rates_sb, 0.0)

            ag_prev = None  # DRAM tile holding gathered spikes of step t-1

            for t in range(T):
                # ---- input currents: accumulating matmuls ----
                sx_t = sp.tile([128, B], F32, tag="SX")
                nc.sync.dma_start(out=sx_t, in_=XT_in[t])
                psum = psp.tile([B, NLOC], F32, tag="PS")
                nc.tensor.matmul(
                    psum, sx_t, win_sb, start=True, stop=(ag_prev is None)
                )
                if ag_prev is not None:
                    s_t = sp.tile([128, GCHUNK * B], MM_DT, tag="S")
                    for d in range(NCORES):
                        nc.sync.dma_start(
                            out=s_t[:, d * NCHUNK * B : (d + 1) * NCHUNK * B],
                            in_=ag_prev[d * 128 : (d + 1) * 128, :],
                        )
                    for k in range(GCHUNK):
                        nc.tensor.matmul(
                            psum,
                            s_t[:, k * B : (k + 1) * B],
                            w_sb[:, k * NLOC : (k + 1) * NLOC],
                            start=False,
                            stop=(k == GCHUNK - 1),
                        )

                # ---- v_dec from prior state (no dependence on psum) ----
                v_dec = sp.tile([B, NLOC], F32, tag="VD")
                nc.vector.scalar_tensor_tensor(
                    out=v_dec[:, :E_LOC], in0=v_sb[:, :E_LOC], scalar=1.0 - TAU_E,
                    in1=u_sb[:, :E_LOC], op0=OP.mult, op1=OP.add,
                )
                nc.vector.scalar_tensor_tensor(
                    out=v_dec[:, E_LOC:], in0=v_sb[:, E_LOC:], scalar=1.0 - TAU_I,
                    in1=u_sb[:, E_LOC:], op0=OP.mult, op1=OP.add,
                )

                # ---- spikes in [n, b] layout: transpose + threshold (bf16) ----
                for j in range(NCHUNK):
                    w = 128 if j < 2 else I_LOC
                    tp = tpp.tile([128, B], F32, tag=f"TP{j}")
                    nc.tensor.transpose(
                        tp[:w, :], v_dec[:, j * 128 : j * 128 + w], ident
                    )
                    # full 128 rows: pad rows get 0/1 garbage that multiplies
                    # zero weight columns (is_gt never yields NaN)
                    nc.vector.tensor_scalar(
                        out=zt_sb[:, j * B : (j + 1) * B], in0=tp[:, :],
                        scalar1=1.0, scalar2=None, op0=OP.is_gt,
                    )

                # ---- exchange spikes (overlaps with everything below) ----
                if 1 <= t <= T - 2:
                    ag_in = dp.tile([128, NCHUNK * B], MM_DT, tag="AGI")
                    ag_out = dp.tile([NCORES * 128, NCHUNK * B], MM_DT, tag="AGO")
                    nc.sync.dma_start(out=ag_in[:, : 2 * B], in_=zt_sb[:, : 2 * B])
                    nc.sync.dma_start(out=ag_in[:, 2 * B :], in_=zt_sb[:, 2 * B :])
                    nc.gpsimd.collective_compute(
                        "AllGather",
                        OP.bypass,
                        replica_groups=rg,
                        ins=[ag_in[:]],
                        outs=[ag_out[:]],
                    )
                    ag_prev = ag_out
                else:
                    ag_prev = None if t == 0 else ag_prev

                # ---- rates accumulation (E rows only) ----
                nc.gpsimd.tensor_tensor(
                    out=rates_sb, in0=rates_sb, in1=_f32(zt_sb[:, : 2 * B]), op=OP.add
                )

                # ---- state updates ----
                # v = (v_dec <= 1) * v_dec
                nc.vector.scalar_tensor_tensor(
                    out=v_sb, in0=v_dec, scalar=1.0, in1=v_dec,
                    op0=OP.is_le, op1=OP.mult,
                )
                # u = 0.8*u + psum   (weights pre-scaled by tau on host)
                nc.vector.scalar_tensor_tensor(
                    out=u_sb, in0=u_sb, scalar=SYN_DEC, in1=psum,
                    op0=OP.mult, op1=OP.add,
                )

            nc.sync.dma_start(out=RATES_out[:], in_=rates_sb[:])

    nc.compile()
    return nc


def _prep_inputs(x, W_ee, W_ie, W_ei, W_ii, W_e_in, W_i_in):
    """Host-side: combined per-core weight matrices (tau-pre-scaled) +
    transposed input."""
    Wee = np.maximum(W_ee, 0).astype(np.float32)
    Wie = np.maximum(W_ie, 0).astype(np.float32)
    Wei = np.maximum(W_ei, 0).astype(np.float32)
    Wii = np.maximum(W_ii, 0).astype(np.float32)

    Ws, Wins = [], []
    for c in range(NCORES):
        Ec = slice(c * E_LOC, (c + 1) * E_LOC)
        Ic = slice(c * I_LOC, (c + 1) * I_LOC)
        Wc = np.zeros((KSRC, NLOC), np.float32)
        for d in range(NCORES):
            base = d * PADLOC
            Epre = slice(d * E_LOC, (d + 1) * E_LOC)
            Ipre = slice(d * I_LOC, (d + 1) * I_LOC)
            Wc[base : base + E_LOC, :E_LOC] = Wee[Ec, Epre].T
            Wc[base : base + E_LOC, E_LOC:] = Wie[Ic, Epre].T
            Wc[base + E_LOC : base + NLOC, :E_LOC] = -Wei[Ec, Ipre].T
            Wc[base + E_LOC : base + NLOC, E_LOC:] = -Wii[Ic, Ipre].T
        Wc[:, :E_LOC] *= TAU_E
        Wc[:, E_LOC:] *= TAU_I
        Ws.append(Wc.astype(MM_NP))

        Wi = np.empty((IN, NLOC), np.float32)
        Wi[:, :E_LOC] = W_e_in[Ec].T * TAU_E
        Wi[:, E_LOC:] = W_i_in[Ic].T * TAU_I
        Wins.append(Wi)

    xT = np.ascontiguousarray(
        np.asarray(x, np.float32).transpose(1, 2, 0)
    )  # [T, IN, B]
    return Ws, Wins, xT


_CACHE = {}


def _get_kernel(T):
    if T not in _CACHE:
        _CACHE[T] = build_kernel(T)
    return _CACHE[T]


def run_spikes(x, W_ee, W_ie, W_ei, W_ii, W_e_in, W_i_in, T=None, trace=False):
    """Run the device portion; returns spike-count sums [B, N_E] and results."""
    T = x.shape[1] if T is None else T
    Ws, Wins, xT = _prep_inputs(x, W_ee, W_ie, W_ei, W_ii, W_e_in, W_i_in)
    xT = xT[:T]
    nc = _get_kernel(T)
    in_maps = [{"W": Ws[c], "WIN": Wins[c], "XT": xT} for c in range(NCORES)]
    res = run_bass_kernel_spmd(
        nc, in_maps, core_ids=list(range(NCORES)), trace=trace
    )
    R = np.stack([res.results[c]["RATES"] for c in range(NCORES)])  # [c,p,2B]
    R = R.reshape(NCORES, 128, 2, B)                   # [c, p, j, b]
    counts = R.transpose(3, 0, 2, 1).reshape(B, N_E)   # [b, c*256+j*128+p]
    return counts, res


def kernel(x, W_ee, W_ie, W_ei, W_ii, W_e_in, W_i_in, readout_w, readout_b):
    counts, _ = run_spikes(x, W_ee, W_ie, W_ei, W_ii, W_e_in, W_i_in)
    rates = counts / np.float32(x.shape[1])
    y = rates.astype(np.float32) @ np.asarray(readout_w, np.float32).T
    return (y + np.asarray(readout_b, np.float32)).astype(np.float32)
